# revision 51
# baseline (speedup 1.0000x reference)
"""Multi-head attention on 8 TRN2 NeuronCores.

Sharding: core c -> (batch b = c // 2, head-group hg = c % 2 of 8 heads).
Each core computes a partial projection output for its batch (its 8 heads'
contribution); the host sums the two head-group partials per batch and adds
b_proj.

Per-core math (all matmul operands bf16, PSUM accumulation f32):
  qT, kT = (w_q^T x^T), (w_k^T x^T)        [inner=512, tok=2048]
  v      = x w_v                           [tok=2048, inner=512]
  scoresT_h = k_h^T^T q_h^T                [ktok, q]; the pair's two heads
                                           run as row-tiled (K=64) concurrent
                                           matmuls (tile rows 0-63 / 64-127)
  expT = exp(scale * scoresT)              ACT engine, no max subtraction
  po   = [v_h0 | v_h1]^T expT              pv pair as col-tiled concurrent
                                           matmuls: po[0:64]=h0, po[64:]=h1
                                           (tile cols 0-63 / 64-127, each
                                           streaming its own exp half)
  den  = ones^T (sum_kl expT)              DVE accumulates exp tiles per
                                           block (bf16, seeded by the first
                                           exp tile); one 1-col matmul
                                           reduces the 128 key partitions;
                                           recip + gpsimd partition-broadcast
  aoT  = po * rbc                          one DVE multiply normalizes both
                                           heads straight into aoT (h1 lands
                                           on partitions 64-127 directly - no
                                           partition-shift DMA)
  y = attn_outT^T w_proj                   [tok, dim] partial, bf16 out
                                           (the qt=3 column is split kk 0-1 /
                                           kk 2-3 so only a 2-matmul finish
                                           trails the final evict)
"""

import numpy as np
import ml_dtypes
from contextlib import ExitStack

B = 4
N = 2048
DIM = 1024
HEADS = 16
HDIM = 64
H_CORE = 8              # heads per core
INNER_C = H_CORE * HDIM  # 512 per-core inner dim
SCALE = HDIM ** -0.5
NCORES = 8

KD = DIM // 128          # 8 contraction tiles over model dim
MT = INNER_C // 128      # 4 inner tiles (head pairs)
NT = N // 512            # 4 query tiles of 512
VT = N // 128            # 16 key tiles of 128
PT = INNER_C // 128      # 4 proj contraction tiles
PVB = 8                  # pv batch period (slots); batching pv matmuls cuts
                         # the PE row/col-tile config switches (each exposed
                         # LDWEIGHTS ~107ns) from one per 2 slots to one per
                         # batch
PVLAG = 8                # pv batch at slot s2 covers pv slots <= s2 - PVLAG

_NC_CACHE = {}


def _build_nc(debug=False):
    import concourse.bass as bass
    import concourse.tile as tile
    from concourse import bacc, mybir

    f32 = mybir.dt.float32
    bf16 = mybir.dt.bfloat16
    AF = mybir.ActivationFunctionType

    nc = bacc.Bacc("TRN2", target_bir_lowering=False, debug=False)

    # weights arrive pre-rearranged from the host so every DMA row is
    # contiguous (2KB/partition descriptors; the (kk p)->p rearrange done
    # on-device costs 256B strided descriptors at ~9.5GB/s per queue)
    xT = nc.dram_tensor("xT", [DIM, N], bf16, kind="ExternalInput").ap()
    wq = nc.dram_tensor("wq", [128, MT, KD, 128], bf16, kind="ExternalInput").ap()
    wk = nc.dram_tensor("wk", [128, MT, KD, 128], bf16, kind="ExternalInput").ap()
    wv = nc.dram_tensor("wv", [128, MT, KD, 128], bf16, kind="ExternalInput").ap()
    wp = nc.dram_tensor("wp", [128, PT, DIM], bf16, kind="ExternalInput").ap()
    out = nc.dram_tensor("out", [N, DIM], bf16, kind="ExternalOutput").ap()
    dbg = {}
    if debug:
        dbg["qT"] = nc.dram_tensor("d_qT", [128, MT, N], bf16, kind="ExternalOutput").ap()
        dbg["kT"] = nc.dram_tensor("d_kT", [128, MT, N], bf16, kind="ExternalOutput").ap()
        dbg["v"] = nc.dram_tensor("d_v", [128, VT, H_CORE, HDIM], bf16, kind="ExternalOutput").ap()
        dbg["ex"] = nc.dram_tensor("d_ex", [2, 128, VT, 512], bf16, kind="ExternalOutput").ap()
        dbg["po"] = nc.dram_tensor("d_po", [128, 512], f32, kind="ExternalOutput").ap()
        dbg["den"] = nc.dram_tensor("d_den", [128, 512], f32, kind="ExternalOutput").ap()
        dbg["rbc"] = nc.dram_tensor("d_rbc", [128, 512], f32, kind="ExternalOutput").ap()
        dbg["aoT"] = nc.dram_tensor("d_aoT", [128, PT, N], bf16, kind="ExternalOutput").ap()

    with tile.TileContext(nc) as tc, ExitStack() as ctx:
        big = ctx.enter_context(tc.tile_pool(name="big", bufs=1))
        exp_pool = ctx.enter_context(tc.tile_pool(name="exp", bufs=17))
        den_pool = ctx.enter_context(tc.tile_pool(name="den", bufs=4))
        small = ctx.enter_context(tc.tile_pool(name="small", bufs=4))
        # PSUM budget (8 banks): mm 2x1 + scores 2x2 + pv 2x1 = 8
        mm_psum = ctx.enter_context(tc.tile_pool(name="mmps", bufs=2, space="PSUM"))
        sc_psum = ctx.enter_context(tc.tile_pool(name="scps", bufs=2, space="PSUM"))
        pv_psum = ctx.enter_context(tc.tile_pool(name="pvps", bufs=2, space="PSUM"))

        # ---- persistent SBUF tensors ----
        xT_s = big.tile([128, KD, N], bf16)          # x^T tiled over dim
        wq_s = big.tile([128, MT, KD, 128], bf16)    # [p, m, kk, col]
        wk_s = big.tile([128, MT, KD, 128], bf16)
        wv_s = big.tile([128, MT, KD, 128], bf16)
        wp_s = big.tile([128, PT, DIM], bf16)
        qT_s = big.tile([128, MT, N], bf16)          # [inner(pair), tok]
        kT_s = big.tile([128, MT, N], bf16)
        v_s = big.tile([128, VT, H_CORE, HDIM], bf16)  # [tok, h, d]
        aoT_s = big.tile([128, PT, N], bf16)         # attn_out^T [inner(pair), tok]
        ones_s = big.tile([128, 1], bf16)            # den-reduction lhsT

        # ---- input DMAs, in first-use order, all contiguous-row pieces ----
        # the upfront k(0,0)/q(0,0) HALF chunks (kk 0-3) need only quarter
        # slices of wk/wq m=0 and the kk 0-3 part of the n=0 xT slice; land
        # those first so the first real matmul starts ~3us earlier
        nc.sync.dma_start(out=wk_s[:, 0, 0:4], in_=wk[:, 0, 0:4])
        nc.sync.dma_start(out=wq_s[:, 0, 0:4], in_=wq[:, 0, 0:4])
        for kk in range(4):
            nc.sync.dma_start(
                out=xT_s[:, kk, 0:512], in_=xT[kk * 128:(kk + 1) * 128, 0:512])
        nc.sync.dma_start(out=wk_s[:, 0, 4:8], in_=wk[:, 0, 4:8])
        nc.sync.dma_start(out=wq_s[:, 0, 4:8], in_=wq[:, 0, 4:8])
        for kk in range(4, KD):
            nc.sync.dma_start(
                out=xT_s[:, kk, 0:512], in_=xT[kk * 128:(kk + 1) * 128, 0:512])
        # xT n=1..2 next: k(0,n) chunks consume them from slot 4n on, which
        # is tighter than anything wv / wk m>0 feeds; wv before xT n=3 so
        # the first v chunks (slot ~10) don't stall on it
        for n in range(1, NT):
            for kk in range(KD):
                nc.sync.dma_start(
                    out=xT_s[:, kk, n * 512:(n + 1) * 512],
                    in_=xT[kk * 128:(kk + 1) * 128, n * 512:(n + 1) * 512])
            if n == 1:
                nc.sync.dma_start(out=wq_s[:, 1], in_=wq[:, 1])
            elif n == 2:
                for m in range(MT):
                    nc.sync.dma_start(out=wv_s[:, m], in_=wv[:, m])
        for m in range(1, MT):
            nc.sync.dma_start(out=wk_s[:, m], in_=wk[:, m])
        nc.sync.dma_start(out=wq_s[:, 2], in_=wq[:, 2])
        nc.sync.dma_start(out=wq_s[:, 3], in_=wq[:, 3])
        for kk in range(PT):
            nc.sync.dma_start(out=wp_s[:, kk], in_=wp[:, kk])
        # warm the PE HAM clock-gate (~3.4us of sustained activity releases
        # the 1.2GHz throttle) with dummy matmuls while the input DMAs
        # stream. wsrc memset FIRST so the chain starts ~0.6us in; 8 cold
        # matmuls (~3.4us) end right as the first chunk's DMA lands (~4.8us)
        # - a longer chain would head-block it in the in-order PE queue
        wsrc = small.tile([128, 512], bf16, tag="wsrc", bufs=1)
        nc.vector.memset(wsrc[:, :], 0.0)
        wps_ = mm_psum.tile([128, 512], f32, tag="mm", name="warm_ps")
        for _ in range(8):
            nc.tensor.matmul(wps_[:, :], lhsT=wsrc[:, 0:128],
                             rhs=wsrc[:, :], start=True, stop=True)
        nc.vector.memset(ones_s[:, :], 1.0)
        # touch Exp early so the ~2.7us ACT table load hides under input DMA
        warm = small.tile([1, 2], f32, tag="warm")
        nc.vector.memset(warm[:, :], 0.0)
        nc.scalar.activation(warm[:, :], warm[:, :], AF.Exp, scale=1.0)

        # ---- chunk emitters (fillers) ----
        # k/q/v chunks are emitted in two 4-MM halves so a single filler unit
        # never delays the next slot's scores by more than ~1us
        live_ps = {}

        def qkv_chunk(w_s, o_s, key, m, n, half):
            if half == 0:
                ps = mm_psum.tile([128, 512], f32, tag="mm", name=f"mm{key}_{m}_{n}")
                live_ps[(key, m, n)] = ps
            else:
                ps = live_ps.pop((key, m, n))
            for kk in range(4 * half, 4 * half + 4):
                nc.tensor.matmul(
                    ps[:, :],
                    lhsT=w_s[:, m, kk, :],
                    rhs=xT_s[:, kk, n * 512:(n + 1) * 512],
                    start=(kk == 0),
                    stop=(kk == KD - 1),
                )
            if half == 1:
                nc.vector.tensor_copy(o_s[:, m, n * 512:(n + 1) * 512], ps[:, :])

        def v_chunk(t, half):
            if half == 0:
                ps = mm_psum.tile([128, 512], f32, tag="mm", name=f"mmv_{t}")
                live_ps[("v", t)] = ps
            else:
                ps = live_ps.pop(("v", t))
            for kk in range(4 * half, 4 * half + 4):
                nc.tensor.matmul(
                    ps[:, :],
                    lhsT=xT_s[:, kk, t * 128:(t + 1) * 128],
                    rhs=wv_s[:, :, kk, :],
                    start=(kk == 0),
                    stop=(kk == KD - 1),
                )
            if half == 1:
                nc.vector.tensor_copy(
                    v_s[:, t, :, :],
                    ps.rearrange("p (h d) -> p h d", h=H_CORE),
                )

        def proj_chunk(qt, mt, n):
            tok0 = qt * 512 + mt * 128
            ps = mm_psum.tile([128, 512], f32, tag="mm", name=f"mmp_{qt}_{mt}_{n}")
            for kk in range(PT):
                nc.tensor.matmul(
                    ps[:, :],
                    lhsT=aoT_s[:, kk, tok0:tok0 + 128],
                    rhs=wp_s[:, kk, n * 512:(n + 1) * 512],
                    start=(kk == 0),
                    stop=(kk == PT - 1),
                )
            y_t = small.tile([128, 512], bf16, tag="yt")
            nc.vector.tensor_copy(y_t[:, :], ps[:, :])
            nc.sync.dma_start(
                out=out[tok0:tok0 + 128, n * 512:(n + 1) * 512],
                in_=y_t[:, :],
            )

        # the last qt column's proj is split: part A (kk 0-1, available
        # after evict(1,qt3) at s2=238) fills the late-loop slots where all
        # other fillers have run out; part B (kk 2-3 + add) is the only
        # work left after the final evict, shortening the tail
        ya_tiles = {}

        def proj_a(qt, mt, n):
            tok0 = qt * 512 + mt * 128
            ps = mm_psum.tile([128, 512], f32, tag="mm", name=f"mpa_{mt}_{n}")
            for kk in range(2):
                nc.tensor.matmul(
                    ps[:, :],
                    lhsT=aoT_s[:, kk, tok0:tok0 + 128],
                    rhs=wp_s[:, kk, n * 512:(n + 1) * 512],
                    start=(kk == 0),
                    stop=(kk == 1),
                )
            ya = small.tile([128, 512], bf16, tag="ya", bufs=8,
                            name=f"ya_{mt}_{n}")
            nc.vector.tensor_copy(ya[:, :], ps[:, :])
            ya_tiles[(mt, n)] = ya

        def proj_b(qt, mt, n):
            tok0 = qt * 512 + mt * 128
            ps = mm_psum.tile([128, 512], f32, tag="mm", name=f"mpb_{mt}_{n}")
            for kk in range(2, PT):
                nc.tensor.matmul(
                    ps[:, :],
                    lhsT=aoT_s[:, kk, tok0:tok0 + 128],
                    rhs=wp_s[:, kk, n * 512:(n + 1) * 512],
                    start=(kk == 2),
                    stop=(kk == PT - 1),
                )
            y_t = small.tile([128, 512], bf16, tag="yt")
            nc.vector.tensor_add(y_t[:, :], ps[:, :], ya_tiles[(mt, n)][:, :])
            nc.sync.dma_start(
                out=out[tok0:tok0 + 128, n * 512:(n + 1) * 512],
                in_=y_t[:, :],
            )

        # ---- attention stream ----
        # Block order: super-rows of qt pairs for the first half (halves the
        # early k-chunk pressure: pair g enters at block 2g), then qt-major
        # for qt=2,3 so proj(2) has in-stream room and only proj(3) trails.
        blocks = ([(g, dq) for g in range(MT) for dq in (0, 1)]
                  + [(g, 2) for g in range(MT)] + [(g, 3) for g in range(MT)])
        NBLK = len(blocks)
        bstate = {bi: {"exs": [], "da": None, "rbc": None, "po": None}
                  for bi in range(NBLK)}

        def sc_exp(bi, kt):
            g, qt = blocks[bi]
            st = bstate[bi]
            qsl = slice(qt * 512, (qt + 1) * 512)
            ksl = slice(kt * 128, (kt + 1) * 128)
            ps = sc_psum.tile([128, 1024], f32, tag="sc", name=f"sc_{bi}_{kt}")
            nc.tensor.matmul(ps[:, 0:512], lhsT=kT_s[0:64, g, ksl],
                             rhs=qT_s[0:64, g, qsl], start=True, stop=True)
            nc.tensor.matmul(ps[:, 512:1024], lhsT=kT_s[64:128, g, ksl],
                             rhs=qT_s[64:128, g, qsl], start=True, stop=True)
            ex = exp_pool.tile([128, 2, 512], bf16, tag="ex", name=f"ex_{bi}_{kt}")
            nc.scalar.activation(
                ex.rearrange("p h q -> p (h q)"), ps[:, :], AF.Exp,
                scale=SCALE)
            st["exs"].append(ex)
            if debug and bi == 0:
                nc.sync.dma_start(out=dbg["ex"][0][:, kt, :], in_=ex[:, 0, :])
                nc.sync.dma_start(out=dbg["ex"][1][:, kt, :], in_=ex[:, 1, :])

        def den_add(s):
            # denominator partial: da += ex on DVE (bf16 2x mode). Emitted 2
            # slots after its sc_exp so the add never waits on its exp when
            # it reaches the head of the in-order DVE queue (that would
            # delay the psum-freeing casts/muls queued behind it). kt==0
            # seeds the chain with the exp tile itself - no copy needed.
            bi, kt = divmod(s, VT)
            st = bstate[bi]
            exf = st["exs"][kt].rearrange("p h q -> p (h q)")
            if kt == 0:
                st["da"] = exf
                return
            da_new = den_pool.tile([128, 1024], bf16, tag="da",
                                   name=f"da_{bi}_{kt}")
            nc.vector.tensor_add(da_new[:, :], st["da"][:, :], exf)
            st["da"] = da_new

        def den_reduce(bi):
            # ones^T @ da -> den row per head (both on partition 0: DVE
            # lanes cannot shift partitions, so a reciprocal must read the
            # partition it writes); recip; gpsimd-broadcast into rbc halves
            st = bstate[bi]
            da = st["da"]
            dm0 = mm_psum.tile([128, 512], f32, tag="mm", name=f"dmm0_{bi}")
            dm1 = mm_psum.tile([128, 512], f32, tag="mm", name=f"dmm1_{bi}")
            nc.tensor.matmul(dm0[0:1, 0:512], lhsT=ones_s[:, :],
                             rhs=da[:, 0:512], start=True, stop=True)
            nc.tensor.matmul(dm1[0:1, 0:512], lhsT=ones_s[:, :],
                             rhs=da[:, 512:1024], start=True, stop=True)
            r0 = small.tile([1, 512], f32, tag="r0", bufs=2, name=f"r0_{bi}")
            r1 = small.tile([1, 512], f32, tag="r1", bufs=2, name=f"r1_{bi}")
            nc.vector.reciprocal_approx_fast(r0[:, :], dm0[0:1, 0:512])
            nc.vector.reciprocal_approx_fast(r1[:, :], dm1[0:1, 0:512])
            rbc = den_pool.tile([128, 512], f32, tag="rbc", bufs=2,
                                name=f"rbc_{bi}")
            nc.gpsimd.partition_broadcast(rbc[0:64, :], r0[:, :])
            # partition_broadcast requires a dst starting at partition 0 (a
            # base-64 dst lands constant garbage on HW): broadcast h1 into a
            # base-0 staging tile and DMA-shift it up (off the critical path)
            bstg = den_pool.tile([64, 512], f32, tag="bstg", bufs=2,
                                 name=f"bstg_{bi}")
            nc.gpsimd.partition_broadcast(bstg[:, :], r1[:, :])
            nc.sync.dma_start(out=rbc[64:128, :], in_=bstg[:, :])
            st["rbc"] = rbc
            if debug and bi == 0:
                dstg = small.tile([128, 512], f32, tag="dstg")
                dstg2 = small.tile([128, 512], f32, tag="dstg")
                nc.vector.tensor_copy(dstg[0:1, :], dm0[0:1, :])
                nc.vector.tensor_copy(dstg2[0:1, :], dm1[0:1, :])
                nc.sync.dma_start(out=dbg["den"][0:1, :], in_=dstg[0:1, :])
                nc.sync.dma_start(out=dbg["den"][64:65, :], in_=dstg2[0:1, :])

        def pv_slot(bi, kl):
            g, qt = blocks[bi]
            st = bstate[bi]
            if st["po"] is None:
                st["po"] = pv_psum.tile([128, 512], f32, tag="pv",
                                        name=f"po_{bi}")
            po = st["po"]
            ex = st["exs"][kl]
            stt = kl == 0
            stp = kl == VT - 1
            nc.tensor.matmul(po[0:64, :], lhsT=v_s[:, kl, 2 * g, :],
                             rhs=ex[:, 0, :], start=stt, stop=stp,
                             tile_position=(0, 0))
            nc.tensor.matmul(po[64:128, :], lhsT=v_s[:, kl, 2 * g + 1, :],
                             rhs=ex[:, 1, :], start=stt, stop=stp,
                             tile_position=(0, 64))

        def pv_evict(bi):
            g, qt = blocks[bi]
            st = bstate[bi]
            if debug and bi == 0:
                pstg = small.tile([128, 512], f32, tag="dstg")
                nc.vector.tensor_copy(pstg[:, :], st["po"][:, :])
                nc.sync.dma_start(out=dbg["po"], in_=pstg[:, :])
                nc.sync.dma_start(out=dbg["rbc"], in_=st["rbc"][:, :])
            nc.vector.tensor_mul(
                aoT_s[:, g, qt * 512:(qt + 1) * 512],
                st["po"][:, :], st["rbc"][:, :])
            st["exs"] = None
            st["po"] = None
            st["da"] = None

        # ---- deadline-driven filler schedule ----
        # unit = (earliest_slot, deadline_slot, fn); at slot s all units with
        # deadline <= s are emitted (program order precedes consumers), and
        # eligible units are pulled forward to keep a steady emission rate.
        units = []

        def block_slot(g, qt):
            idx = blocks.index((g, qt))
            return idx * VT

        # full 8-MM chunks as single units: the scheduler keeps
        # adjacent-priority matmuls together when deps allow, halving the
        # filler<->sc/pv config-switch boundaries (~107ns exposed LDWEIGHTS
        # each); per-kk DMA deps still let half-0 start before half-1's
        # xT slices land
        def add_kq(w_s, o_s, key, g, qt_or_n, d):
            def emit(w_s=w_s, o_s=o_s, key=key, g=g, n=qt_or_n):
                qkv_chunk(w_s, o_s, key, g, n, 0)
                qkv_chunk(w_s, o_s, key, g, n, 1)
            units.append((0, d, emit))

        # k chunks: k(g,n) feeds sc at slot block_slot(g,0) + 4n; give ~5
        # slots of margin so the consumer never queues right behind it
        for g in range(MT):
            for n in range(NT):
                if g == 0 and n == 0:
                    continue  # upfront
                d = block_slot(g, 0) + 4 * n - 1 if g == 0 else (
                    block_slot(g, 0) + 4 * n - 5)
                add_kq(wk_s, kT_s, "k", g, n, d)
        # q chunks: q(g,qt) before its block starts, with margin
        for g in range(MT):
            for qt in range(NT):
                if g == 0 and qt == 0:
                    continue  # upfront
                d = max(block_slot(g, qt) - 5, 8)
                add_kq(wq_s, qT_s, "q", g, qt, d)
        # v chunks: v(t) is consumed by the pv batch at slot-pair
        # s2v(t) = 6 + 8*ceil((t+2)/8) (the first s2 = 6 mod 8 with
        # s2 - PVLAG >= t); emit it one pair earlier
        for t in range(VT):
            s2v = 6 + PVB * ((t + PVLAG - 6 + PVB - 1) // PVB)
            # earliest 10 for the first v chunks: the wv DMA lands ~slot 9
            # and an eagerly pulled v chunk would stall the PE on it
            def emit_v(t=t):
                v_chunk(t, 0)
                v_chunk(t, 1)
            units.append((10 if t <= 6 else 0, s2v - 2, emit_v))
        # proj chunks: after the last evict of their qt column; evict(bi) is
        # emitted in the pv batch at s2 = VT*bi + 30
        pearliest = {}
        for qt in range(NT):
            bi_last = blocks.index((MT - 1, qt))
            pearliest[qt] = VT * bi_last + 30
        pdeadline = {0: 160, 1: 184, 2: 218}
        for qt in range(NT - 1):
            for i, (mt, n) in enumerate(
                    [(mt, n) for mt in range(PT) for n in range(2)]):
                units.append((pearliest[qt], pdeadline[qt] + 2 * i,
                              (lambda q_, m_, n_: lambda: proj_chunk(
                                  q_, m_, n_))(qt, mt, n)))
        # qt=3 part A: kk 0-1 need evict(0,3) (s2=222) / evict(1,3) (s2=238)
        for i, (mt, n) in enumerate(
                [(mt, n) for mt in range(PT) for n in range(2)]):
            units.append((238, 240 + 2 * i,
                          (lambda m_, n_: lambda: proj_a(3, m_, n_))(mt, n)))
        # qt=3 part B: tail only (needs the final evict)
        for mt in range(PT):
            for n in range(2):
                units.append((10 ** 9, 10 ** 9,
                              (lambda m_, n_: lambda: proj_b(3, m_, n_))(mt, n)))

        units.sort(key=lambda u: u[1])
        n_stream_units = sum(1 for u in units if u[1] < 10 ** 8)
        NSLOT = NBLK * VT

        # upfront: k(0,0) + q(0,0)
        for h in (0, 1):
            qkv_chunk(wk_s, kT_s, "k", 0, 0, h)
            qkv_chunk(wq_s, qT_s, "q", 0, 0, h)

        pv_ptr = [0]

        def emit_pv_upto(limit):
            while pv_ptr[0] <= limit:
                pbi, pkl = divmod(pv_ptr[0], VT)
                pv_slot(pbi, pkl)
                if pkl == VT - 1:
                    pv_evict(pbi)
                pv_ptr[0] += 1

        emitted = [0]

        def emit_fillers(s):
            # 1) everything overdue (deadline <= s), in deadline order;
            # 2) then pull eligible units forward to the linear ramp so late
            #    slack (proj columns) spreads instead of bunching
            i = 0
            while i < len(units):
                e0, d0, fn = units[i]
                if d0 <= s:
                    assert e0 <= s, f"unit overdue before eligible: {e0} {d0} {s}"
                    units.pop(i)
                    fn()
                    emitted[0] += 1
                else:
                    i += 1
            ramp = ((s + 1) * n_stream_units + NSLOT - 1) // NSLOT
            i = 0
            while emitted[0] < ramp and i < len(units):
                e0, d0, fn = units[i]
                if e0 <= s:
                    units.pop(i)
                    fn()
                    emitted[0] += 1
                else:
                    i += 1

        # Slots are emitted in PAIRS of score matmuls (the second pair's
        # LDWEIGHTS hide inside the first's stream since the 64-row tile
        # config doesn't change), with pv matmuls BATCHED once per PVB slots
        # so the PE pays the row/col-tile config-switch tax (~107ns exposed
        # LDWEIGHTS + drain) once per batch instead of once per pair.
        # den_reduce(bi) is deferred into the next block (kt==2), emitted
        # BEFORE the pair's scores: its matmuls' deps are ready (block ended
        # 2 slots ago) and its reciprocals land at the DVE queue head,
        # releasing the mm psum bufs quickly. The evict that consumes rbc is
        # emitted in the pv batch at s2 = VT*bi + 30, leaving ~12 slots for
        # the den-MM -> recip -> broadcast -> dma chain.
        for s2 in range(0, NBLK * VT, 2):
            for s in (s2, s2 + 1):
                bi, kt = divmod(s, VT)
                sc_exp(bi, kt)
            if s2 % PVB == 6:
                emit_pv_upto(s2 - PVLAG)
            for s in (s2 - 2, s2 - 1):
                if s >= 0:
                    den_add(s)
            emit_fillers(s2)
            # den matmuls amid the fillers: same full-128-row config, so
            # they extend a filler run instead of opening their own
            # (saving a ~107ns LDW exposure on re-entry)
            if s2 % VT == 2 and s2 >= VT:
                den_reduce(s2 // VT - 1)
            emit_fillers(s2 + 1)
        den_add(NBLK * VT - 2)
        den_add(NBLK * VT - 1)
        # a few always-ready dummies in the free pv-pool slot: the PE would
        # otherwise idle ~1.3us between the last score stream and the final
        # den matmuls (which wait on the last exp -> DVE adds)
        pvd = pv_psum.tile([128, 512], f32, tag="pv", name="tail_warm")
        for _ in range(8):
            nc.tensor.matmul(pvd[:, :], lhsT=wsrc[:, 0:128],
                             rhs=wsrc[:, :], start=True, stop=True)
        den_reduce(NBLK - 1)
        emit_pv_upto(NBLK * VT - 1)
        # bridge the den->recip->broadcast->evict chain (~4us) with dummy
        # matmuls so the PE HAM clock doesn't re-throttle before the final
        # projection burst
        # 20 matmuls (~4.3us warm): the den(15)->recip->broadcast->dma->mul
        # chain is ~6.8us past the last exp; a shorter bridge lets the PE
        # idle >3.4us, HAM re-throttles, and the 16 part-B matmuls run at
        # 1.2GHz instead of 2.4
        wps2 = mm_psum.tile([128, 512], f32, tag="mm", name="warm_ps2")
        for _ in range(20):
            nc.tensor.matmul(wps2[:, :], lhsT=wsrc[:, 0:128],
                             rhs=wsrc[:, :], start=True, stop=True)
        # tail: anything left (proj of the last column)
        for _, _, fn in units:
            fn()

        if debug:
            nc.sync.dma_start(out=dbg["qT"], in_=qT_s[:, :, :])
            nc.sync.dma_start(out=dbg["kT"], in_=kT_s[:, :, :])
            nc.sync.dma_start(out=dbg["v"], in_=v_s[:, :, :, :])
            nc.sync.dma_start(out=dbg["aoT"], in_=aoT_s[:, :, :])

    nc.compile()
    return nc


def _get_nc():
    if "nc" not in _NC_CACHE:
        _NC_CACHE["nc"] = _build_nc()
    return _NC_CACHE["nc"]


def _prep_inputs(x, w_qkv, w_proj):
    bf16 = ml_dtypes.bfloat16
    x = np.asarray(x, dtype=np.float32)
    w_qkv = np.asarray(w_qkv, dtype=np.float32)
    w_proj = np.asarray(w_proj, dtype=np.float32)

    w3 = w_qkv.reshape(DIM, 3, HEADS, HDIM)
    wp4 = w_proj.reshape(HEADS, HDIM, DIM)

    def wlay(w):
        # [DIM, INNER_C] -> [128p, MT, KD, 128c] so each on-device DMA row
        # (per partition, per m) is 2KB contiguous
        return np.ascontiguousarray(
            w.reshape(KD, 128, MT, 128).transpose(1, 2, 0, 3)).astype(bf16)

    in_maps = []
    for c in range(NCORES):
        b, hg = c // 2, c % 2
        hs = slice(hg * H_CORE, (hg + 1) * H_CORE)
        wpm = wp4[hs].reshape(INNER_C, DIM)
        in_maps.append({
            "xT": np.ascontiguousarray(x[b].T).astype(bf16),
            "wq": wlay(w3[:, 0, hs].reshape(DIM, INNER_C)),
            "wk": wlay(w3[:, 1, hs].reshape(DIM, INNER_C)),
            "wv": wlay(w3[:, 2, hs].reshape(DIM, INNER_C)),
            "wp": np.ascontiguousarray(
                wpm.reshape(PT, 128, DIM).transpose(1, 0, 2)).astype(bf16),
        })
    return in_maps


def _ensure_trace_hooks():
    """run_bass_kernel_spmd(trace=True) under axon needs antenv.axon_hooks;
    some images lack it. Install a working shim if possible, else make the
    trace path a no-op so execution never crashes on a missing module."""
    import os
    import sys
    try:
        from antenv.axon_hooks import get_axon_ntff_profile_hook  # noqa: F401
        return
    except ImportError:
        pass
    try:
        import types
        from trn_agent_boot.trn_boot import _ntff_profile_via_ctypes

        mod = types.ModuleType("antenv.axon_hooks")
        _h = [_ntff_profile_via_ctypes("/opt/axon/libaxon_pjrt.so")]
        mod.set_axon_ntff_profile_hook = lambda h: _h.__setitem__(0, h)
        mod.get_axon_ntff_profile_hook = lambda: _h[0]
        sys.modules["antenv.axon_hooks"] = mod
        from concourse import bass_utils
        bass_utils.upload_artifacts = lambda tmpdir: tmpdir
    except Exception:
        os.environ["BASS_NEVER_TRACE"] = "1"


def kernel(x, w_qkv, w_proj, b_proj):
    _ensure_trace_hooks()
    from concourse.bass_utils import run_bass_kernel_spmd

    nc = _get_nc()
    in_maps = _prep_inputs(x, w_qkv, w_proj)
    res = run_bass_kernel_spmd(nc, in_maps, core_ids=list(range(NCORES)))
    b_proj = np.asarray(b_proj, dtype=np.float32)
    out = np.empty((B, N, DIM), dtype=np.float32)
    for b in range(B):
        out[b] = (res.results[2 * b]["out"].astype(np.float32)
                  + res.results[2 * b + 1]["out"].astype(np.float32) + b_proj)
    return out



# revision 52
# speedup vs baseline: 1.0067x; 1.0067x over previous
"""Multi-head attention on 8 TRN2 NeuronCores.

Sharding: core c -> (batch b = c // 2, head-group hg = c % 2 of 8 heads).
Each core computes a partial projection output for its batch (its 8 heads'
contribution); the host sums the two head-group partials per batch and adds
b_proj.

Per-core math (all matmul operands bf16, PSUM accumulation f32):
  qT, kT = (w_q^T x^T), (w_k^T x^T)        [inner=512, tok=2048]
  v      = x w_v                           [tok=2048, inner=512]
  scoresT_h = k_h^T^T q_h^T                [ktok, q]; the pair's two heads
                                           run as row-tiled (K=64) concurrent
                                           matmuls (tile rows 0-63 / 64-127)
  expT = exp(scale * scoresT)              ACT engine, no max subtraction
  po   = [v_h0 | v_h1]^T expT              pv pair as col-tiled concurrent
                                           matmuls: po[0:64]=h0, po[64:]=h1
                                           (tile cols 0-63 / 64-127, each
                                           streaming its own exp half)
  den  = ones^T (sum_kl expT)              DVE accumulates exp tiles per
                                           block (bf16, seeded by the first
                                           exp tile); one 1-col matmul
                                           reduces the 128 key partitions;
                                           recip + gpsimd partition-broadcast
  aoT  = po * rbc                          one DVE multiply normalizes both
                                           heads straight into aoT (h1 lands
                                           on partitions 64-127 directly - no
                                           partition-shift DMA)
  y = attn_outT^T w_proj                   [tok, dim] partial, bf16 out
                                           (the qt=3 column is split kk 0-1 /
                                           kk 2-3 so only a 2-matmul finish
                                           trails the final evict)
"""

import numpy as np
import ml_dtypes
from contextlib import ExitStack

B = 4
N = 2048
DIM = 1024
HEADS = 16
HDIM = 64
H_CORE = 8              # heads per core
INNER_C = H_CORE * HDIM  # 512 per-core inner dim
SCALE = HDIM ** -0.5
NCORES = 8

KD = DIM // 128          # 8 contraction tiles over model dim
MT = INNER_C // 128      # 4 inner tiles (head pairs)
NT = N // 512            # 4 query tiles of 512
VT = N // 128            # 16 key tiles of 128
PT = INNER_C // 128      # 4 proj contraction tiles
PVB = 8                  # pv batch period (slots); batching pv matmuls cuts
                         # the PE row/col-tile config switches (each exposed
                         # LDWEIGHTS ~107ns) from one per 2 slots to one per
                         # batch
PVLAG = 8                # pv batch at slot s2 covers pv slots <= s2 - PVLAG

_NC_CACHE = {}


def _build_nc(debug=False):
    import concourse.bass as bass
    import concourse.tile as tile
    from concourse import bacc, mybir

    f32 = mybir.dt.float32
    bf16 = mybir.dt.bfloat16
    AF = mybir.ActivationFunctionType

    nc = bacc.Bacc("TRN2", target_bir_lowering=False, debug=False)

    # weights arrive pre-rearranged from the host so every DMA row is
    # contiguous (2KB/partition descriptors; the (kk p)->p rearrange done
    # on-device costs 256B strided descriptors at ~9.5GB/s per queue)
    xT = nc.dram_tensor("xT", [DIM, N], bf16, kind="ExternalInput").ap()
    wq = nc.dram_tensor("wq", [128, MT, KD, 128], bf16, kind="ExternalInput").ap()
    wk = nc.dram_tensor("wk", [128, MT, KD, 128], bf16, kind="ExternalInput").ap()
    wv = nc.dram_tensor("wv", [128, MT, KD, 128], bf16, kind="ExternalInput").ap()
    wp = nc.dram_tensor("wp", [128, PT, DIM], bf16, kind="ExternalInput").ap()
    out = nc.dram_tensor("out", [N, DIM], bf16, kind="ExternalOutput").ap()
    dbg = {}
    if debug:
        dbg["qT"] = nc.dram_tensor("d_qT", [128, MT, N], bf16, kind="ExternalOutput").ap()
        dbg["kT"] = nc.dram_tensor("d_kT", [128, MT, N], bf16, kind="ExternalOutput").ap()
        dbg["v"] = nc.dram_tensor("d_v", [128, VT, H_CORE, HDIM], bf16, kind="ExternalOutput").ap()
        dbg["ex"] = nc.dram_tensor("d_ex", [2, 128, VT, 512], bf16, kind="ExternalOutput").ap()
        dbg["po"] = nc.dram_tensor("d_po", [128, 512], f32, kind="ExternalOutput").ap()
        dbg["den"] = nc.dram_tensor("d_den", [128, 512], f32, kind="ExternalOutput").ap()
        dbg["rbc"] = nc.dram_tensor("d_rbc", [128, 512], f32, kind="ExternalOutput").ap()
        dbg["aoT"] = nc.dram_tensor("d_aoT", [128, PT, N], bf16, kind="ExternalOutput").ap()

    with tile.TileContext(nc) as tc, ExitStack() as ctx:
        big = ctx.enter_context(tc.tile_pool(name="big", bufs=1))
        exp_pool = ctx.enter_context(tc.tile_pool(name="exp", bufs=17))
        den_pool = ctx.enter_context(tc.tile_pool(name="den", bufs=4))
        small = ctx.enter_context(tc.tile_pool(name="small", bufs=4))
        # PSUM budget (8 banks): mm 2x1 + scores 2x2 + pv 2x1 = 8
        mm_psum = ctx.enter_context(tc.tile_pool(name="mmps", bufs=2, space="PSUM"))
        sc_psum = ctx.enter_context(tc.tile_pool(name="scps", bufs=2, space="PSUM"))
        pv_psum = ctx.enter_context(tc.tile_pool(name="pvps", bufs=2, space="PSUM"))

        # ---- persistent SBUF tensors ----
        xT_s = big.tile([128, KD, N], bf16)          # x^T tiled over dim
        wq_s = big.tile([128, MT, KD, 128], bf16)    # [p, m, kk, col]
        wk_s = big.tile([128, MT, KD, 128], bf16)
        wv_s = big.tile([128, MT, KD, 128], bf16)
        wp_s = big.tile([128, PT, DIM], bf16)
        qT_s = big.tile([128, MT, N], bf16)          # [inner(pair), tok]
        kT_s = big.tile([128, MT, N], bf16)
        v_s = big.tile([128, VT, H_CORE, HDIM], bf16)  # [tok, h, d]
        aoT_s = big.tile([128, PT, N], bf16)         # attn_out^T [inner(pair), tok]
        ones_s = big.tile([128, 1], bf16)            # den-reduction lhsT

        # ---- input DMAs, in first-use order, all contiguous-row pieces ----
        # the upfront k(0,0)/q(0,0) HALF chunks (kk 0-3) need only quarter
        # slices of wk/wq m=0 and the kk 0-3 part of the n=0 xT slice; land
        # those first so the first real matmul starts ~3us earlier
        nc.sync.dma_start(out=wk_s[:, 0, 0:4], in_=wk[:, 0, 0:4])
        nc.sync.dma_start(out=wq_s[:, 0, 0:4], in_=wq[:, 0, 0:4])
        for kk in range(4):
            nc.sync.dma_start(
                out=xT_s[:, kk, 0:512], in_=xT[kk * 128:(kk + 1) * 128, 0:512])
        nc.sync.dma_start(out=wk_s[:, 0, 4:8], in_=wk[:, 0, 4:8])
        nc.sync.dma_start(out=wq_s[:, 0, 4:8], in_=wq[:, 0, 4:8])
        for kk in range(4, KD):
            nc.sync.dma_start(
                out=xT_s[:, kk, 0:512], in_=xT[kk * 128:(kk + 1) * 128, 0:512])
        # xT n=1..2 next: k(0,n) chunks consume them from slot 4n on, which
        # is tighter than anything wv / wk m>0 feeds; wv before xT n=3 so
        # the first v chunks (slot ~10) don't stall on it
        for n in range(1, NT):
            for kk in range(KD):
                nc.sync.dma_start(
                    out=xT_s[:, kk, n * 512:(n + 1) * 512],
                    in_=xT[kk * 128:(kk + 1) * 128, n * 512:(n + 1) * 512])
            if n == 1:
                nc.sync.dma_start(out=wq_s[:, 1], in_=wq[:, 1])
            elif n == 2:
                for m in range(MT):
                    nc.sync.dma_start(out=wv_s[:, m], in_=wv[:, m])
        for m in range(1, MT):
            nc.sync.dma_start(out=wk_s[:, m], in_=wk[:, m])
        nc.sync.dma_start(out=wq_s[:, 2], in_=wq[:, 2])
        nc.sync.dma_start(out=wq_s[:, 3], in_=wq[:, 3])
        for kk in range(PT):
            nc.sync.dma_start(out=wp_s[:, kk], in_=wp[:, kk])
        # warm the PE HAM clock-gate (~3.4us of sustained activity releases
        # the 1.2GHz throttle) with dummy matmuls while the input DMAs
        # stream. wsrc memset FIRST so the chain starts ~0.6us in; 8 cold
        # matmuls (~3.4us) end right as the first chunk's DMA lands (~4.8us)
        # - a longer chain would head-block it in the in-order PE queue
        wsrc = small.tile([128, 512], bf16, tag="wsrc", bufs=1)
        nc.vector.memset(wsrc[:, :], 0.0)
        wps_ = mm_psum.tile([128, 512], f32, tag="mm", name="warm_ps")
        for _ in range(8):
            nc.tensor.matmul(wps_[:, :], lhsT=wsrc[:, 0:128],
                             rhs=wsrc[:, :], start=True, stop=True)
        nc.vector.memset(ones_s[:, :], 1.0)
        # touch Exp early so the ~2.7us ACT table load hides under input DMA
        warm = small.tile([1, 2], f32, tag="warm")
        nc.vector.memset(warm[:, :], 0.0)
        nc.scalar.activation(warm[:, :], warm[:, :], AF.Exp, scale=1.0)

        # ---- chunk emitters (fillers) ----
        # k/q/v chunks are emitted in two 4-MM halves so a single filler unit
        # never delays the next slot's scores by more than ~1us
        live_ps = {}

        def qkv_chunk(w_s, o_s, key, m, n, half):
            if half == 0:
                ps = mm_psum.tile([128, 512], f32, tag="mm", name=f"mm{key}_{m}_{n}")
                live_ps[(key, m, n)] = ps
            else:
                ps = live_ps.pop((key, m, n))
            for kk in range(4 * half, 4 * half + 4):
                nc.tensor.matmul(
                    ps[:, :],
                    lhsT=w_s[:, m, kk, :],
                    rhs=xT_s[:, kk, n * 512:(n + 1) * 512],
                    start=(kk == 0),
                    stop=(kk == KD - 1),
                )
            if half == 1:
                nc.vector.tensor_copy(o_s[:, m, n * 512:(n + 1) * 512], ps[:, :])

        def v_chunk(t, half):
            if half == 0:
                ps = mm_psum.tile([128, 512], f32, tag="mm", name=f"mmv_{t}")
                live_ps[("v", t)] = ps
            else:
                ps = live_ps.pop(("v", t))
            for kk in range(4 * half, 4 * half + 4):
                nc.tensor.matmul(
                    ps[:, :],
                    lhsT=xT_s[:, kk, t * 128:(t + 1) * 128],
                    rhs=wv_s[:, :, kk, :],
                    start=(kk == 0),
                    stop=(kk == KD - 1),
                )
            if half == 1:
                nc.vector.tensor_copy(
                    v_s[:, t, :, :],
                    ps.rearrange("p (h d) -> p h d", h=H_CORE),
                )

        def proj_chunk(qt, mt, n):
            tok0 = qt * 512 + mt * 128
            ps = mm_psum.tile([128, 512], f32, tag="mm", name=f"mmp_{qt}_{mt}_{n}")
            for kk in range(PT):
                nc.tensor.matmul(
                    ps[:, :],
                    lhsT=aoT_s[:, kk, tok0:tok0 + 128],
                    rhs=wp_s[:, kk, n * 512:(n + 1) * 512],
                    start=(kk == 0),
                    stop=(kk == PT - 1),
                )
            y_t = small.tile([128, 512], bf16, tag="yt")
            nc.vector.tensor_copy(y_t[:, :], ps[:, :])
            nc.sync.dma_start(
                out=out[tok0:tok0 + 128, n * 512:(n + 1) * 512],
                in_=y_t[:, :],
            )

        # the last qt column's proj is split: part A (kk 0-1, available
        # after evict(1,qt3) at s2=238) fills the late-loop slots where all
        # other fillers have run out; part B (kk 2-3 + add) is the only
        # work left after the final evict, shortening the tail
        ya_tiles = {}

        def proj_a(qt, mt, n):
            tok0 = qt * 512 + mt * 128
            ps = mm_psum.tile([128, 512], f32, tag="mm", name=f"mpa_{mt}_{n}")
            for kk in range(2):
                nc.tensor.matmul(
                    ps[:, :],
                    lhsT=aoT_s[:, kk, tok0:tok0 + 128],
                    rhs=wp_s[:, kk, n * 512:(n + 1) * 512],
                    start=(kk == 0),
                    stop=(kk == 1),
                )
            ya = small.tile([128, 512], bf16, tag="ya", bufs=8,
                            name=f"ya_{mt}_{n}")
            nc.vector.tensor_copy(ya[:, :], ps[:, :])
            ya_tiles[(mt, n)] = ya

        def proj_b(qt, mt, n):
            tok0 = qt * 512 + mt * 128
            ps = mm_psum.tile([128, 512], f32, tag="mm", name=f"mpb_{mt}_{n}")
            for kk in range(2, PT):
                nc.tensor.matmul(
                    ps[:, :],
                    lhsT=aoT_s[:, kk, tok0:tok0 + 128],
                    rhs=wp_s[:, kk, n * 512:(n + 1) * 512],
                    start=(kk == 2),
                    stop=(kk == PT - 1),
                )
            y_t = small.tile([128, 512], bf16, tag="yt")
            nc.vector.tensor_add(y_t[:, :], ps[:, :], ya_tiles[(mt, n)][:, :])
            nc.sync.dma_start(
                out=out[tok0:tok0 + 128, n * 512:(n + 1) * 512],
                in_=y_t[:, :],
            )

        # ---- attention stream ----
        # Block order: super-rows of qt pairs for the first half (halves the
        # early k-chunk pressure: pair g enters at block 2g), then qt-major
        # for qt=2,3 so proj(2) has in-stream room and only proj(3) trails.
        blocks = ([(g, dq) for g in range(MT) for dq in (0, 1)]
                  + [(g, 2) for g in range(MT)] + [(g, 3) for g in range(MT)])
        NBLK = len(blocks)
        bstate = {bi: {"exs": [], "da": None, "rbc": None, "po": None}
                  for bi in range(NBLK)}

        def sc_exp(bi, kt):
            g, qt = blocks[bi]
            st = bstate[bi]
            qsl = slice(qt * 512, (qt + 1) * 512)
            ksl = slice(kt * 128, (kt + 1) * 128)
            ps = sc_psum.tile([128, 1024], f32, tag="sc", name=f"sc_{bi}_{kt}")
            nc.tensor.matmul(ps[:, 0:512], lhsT=kT_s[0:64, g, ksl],
                             rhs=qT_s[0:64, g, qsl], start=True, stop=True)
            nc.tensor.matmul(ps[:, 512:1024], lhsT=kT_s[64:128, g, ksl],
                             rhs=qT_s[64:128, g, qsl], start=True, stop=True)
            ex = exp_pool.tile([128, 2, 512], bf16, tag="ex", name=f"ex_{bi}_{kt}")
            nc.scalar.activation(
                ex.rearrange("p h q -> p (h q)"), ps[:, :], AF.Exp,
                scale=SCALE)
            st["exs"].append(ex)
            if debug and bi == 0:
                nc.sync.dma_start(out=dbg["ex"][0][:, kt, :], in_=ex[:, 0, :])
                nc.sync.dma_start(out=dbg["ex"][1][:, kt, :], in_=ex[:, 1, :])

        def den_add(s):
            # denominator partial: da += ex on DVE (bf16 2x mode). Emitted 2
            # slots after its sc_exp so the add never waits on its exp when
            # it reaches the head of the in-order DVE queue (that would
            # delay the psum-freeing casts/muls queued behind it). kt==0
            # seeds the chain with the exp tile itself - no copy needed.
            bi, kt = divmod(s, VT)
            st = bstate[bi]
            exf = st["exs"][kt].rearrange("p h q -> p (h q)")
            if kt == 0:
                st["da"] = exf
                return
            da_new = den_pool.tile([128, 1024], bf16, tag="da",
                                   name=f"da_{bi}_{kt}")
            nc.vector.tensor_add(da_new[:, :], st["da"][:, :], exf)
            st["da"] = da_new

        def den_reduce(bi):
            # ones^T @ da -> den row per head (both on partition 0: DVE
            # lanes cannot shift partitions, so a reciprocal must read the
            # partition it writes); recip; gpsimd-broadcast into rbc halves.
            # The den psum comes from the PV pool's free slot (po(bi-1) was
            # evicted ~4 slots ago, po(bi+1) isn't allocated for another
            # ~12), sequentially per head so one slot suffices - taking mm
            # tiles here would starve the filler chunks of both their
            # accumulator banks for the duration of the den chain
            st = bstate[bi]
            da = st["da"]
            dm0 = pv_psum.tile([128, 512], f32, tag="pv", name=f"dmm0_{bi}")
            nc.tensor.matmul(dm0[0:1, 0:512], lhsT=ones_s[:, :],
                             rhs=da[:, 0:512], start=True, stop=True)
            r0 = small.tile([1, 512], f32, tag="r0", bufs=2, name=f"r0_{bi}")
            r1 = small.tile([1, 512], f32, tag="r1", bufs=2, name=f"r1_{bi}")
            nc.vector.reciprocal_approx_fast(r0[:, :], dm0[0:1, 0:512])
            dm1 = pv_psum.tile([128, 512], f32, tag="pv", name=f"dmm1_{bi}")
            nc.tensor.matmul(dm1[0:1, 0:512], lhsT=ones_s[:, :],
                             rhs=da[:, 512:1024], start=True, stop=True)
            nc.vector.reciprocal_approx_fast(r1[:, :], dm1[0:1, 0:512])
            rbc = den_pool.tile([128, 512], f32, tag="rbc", bufs=2,
                                name=f"rbc_{bi}")
            nc.gpsimd.partition_broadcast(rbc[0:64, :], r0[:, :])
            # partition_broadcast requires a dst starting at partition 0 (a
            # base-64 dst lands constant garbage on HW): broadcast h1 into a
            # base-0 staging tile and DMA-shift it up (off the critical path)
            bstg = den_pool.tile([64, 512], f32, tag="bstg", bufs=2,
                                 name=f"bstg_{bi}")
            nc.gpsimd.partition_broadcast(bstg[:, :], r1[:, :])
            nc.sync.dma_start(out=rbc[64:128, :], in_=bstg[:, :])
            st["rbc"] = rbc
            if debug and bi == 0:
                dstg = small.tile([128, 512], f32, tag="dstg")
                dstg2 = small.tile([128, 512], f32, tag="dstg")
                nc.vector.tensor_copy(dstg[0:1, :], dm0[0:1, :])
                nc.vector.tensor_copy(dstg2[0:1, :], dm1[0:1, :])
                nc.sync.dma_start(out=dbg["den"][0:1, :], in_=dstg[0:1, :])
                nc.sync.dma_start(out=dbg["den"][64:65, :], in_=dstg2[0:1, :])

        def pv_slot(bi, kl):
            g, qt = blocks[bi]
            st = bstate[bi]
            if st["po"] is None:
                st["po"] = pv_psum.tile([128, 512], f32, tag="pv",
                                        name=f"po_{bi}")
            po = st["po"]
            ex = st["exs"][kl]
            stt = kl == 0
            stp = kl == VT - 1
            nc.tensor.matmul(po[0:64, :], lhsT=v_s[:, kl, 2 * g, :],
                             rhs=ex[:, 0, :], start=stt, stop=stp,
                             tile_position=(0, 0))
            nc.tensor.matmul(po[64:128, :], lhsT=v_s[:, kl, 2 * g + 1, :],
                             rhs=ex[:, 1, :], start=stt, stop=stp,
                             tile_position=(0, 64))

        def pv_evict(bi):
            g, qt = blocks[bi]
            st = bstate[bi]
            if debug and bi == 0:
                pstg = small.tile([128, 512], f32, tag="dstg")
                nc.vector.tensor_copy(pstg[:, :], st["po"][:, :])
                nc.sync.dma_start(out=dbg["po"], in_=pstg[:, :])
                nc.sync.dma_start(out=dbg["rbc"], in_=st["rbc"][:, :])
            nc.vector.tensor_mul(
                aoT_s[:, g, qt * 512:(qt + 1) * 512],
                st["po"][:, :], st["rbc"][:, :])
            st["exs"] = None
            st["po"] = None
            st["da"] = None

        # ---- deadline-driven filler schedule ----
        # unit = (earliest_slot, deadline_slot, fn); at slot s all units with
        # deadline <= s are emitted (program order precedes consumers), and
        # eligible units are pulled forward to keep a steady emission rate.
        units = []

        def block_slot(g, qt):
            idx = blocks.index((g, qt))
            return idx * VT

        # full 8-MM chunks as single units: the scheduler keeps
        # adjacent-priority matmuls together when deps allow, halving the
        # filler<->sc/pv config-switch boundaries (~107ns exposed LDWEIGHTS
        # each); per-kk DMA deps still let half-0 start before half-1's
        # xT slices land
        def add_kq(w_s, o_s, key, g, qt_or_n, d):
            def emit(w_s=w_s, o_s=o_s, key=key, g=g, n=qt_or_n):
                qkv_chunk(w_s, o_s, key, g, n, 0)
                qkv_chunk(w_s, o_s, key, g, n, 1)
            units.append((0, d, emit))

        # k chunks: k(g,n) feeds sc at slot block_slot(g,0) + 4n; give ~5
        # slots of margin so the consumer never queues right behind it
        for g in range(MT):
            for n in range(NT):
                if g == 0 and n == 0:
                    continue  # upfront
                d = block_slot(g, 0) + 4 * n - 1 if g == 0 else (
                    block_slot(g, 0) + 4 * n - 5)
                add_kq(wk_s, kT_s, "k", g, n, d)
        # q chunks: q(g,qt) before its block starts, with margin
        for g in range(MT):
            for qt in range(NT):
                if g == 0 and qt == 0:
                    continue  # upfront
                d = max(block_slot(g, qt) - 5, 8)
                add_kq(wq_s, qT_s, "q", g, qt, d)
        # v chunks: v(t) is consumed by the pv batch at slot-pair
        # s2v(t) = 6 + 8*ceil((t+2)/8) (the first s2 = 6 mod 8 with
        # s2 - PVLAG >= t); emit it one pair earlier
        for t in range(VT):
            s2v = 6 + PVB * ((t + PVLAG - 6 + PVB - 1) // PVB)
            # earliest 10 for the first v chunks: the wv DMA lands ~slot 9
            # and an eagerly pulled v chunk would stall the PE on it
            def emit_v(t=t):
                v_chunk(t, 0)
                v_chunk(t, 1)
            units.append((10 if t <= 6 else 0, s2v - 2, emit_v))
        # proj chunks: after the last evict of their qt column; evict(bi) is
        # emitted in the pv batch at s2 = VT*bi + 30
        pearliest = {}
        for qt in range(NT):
            bi_last = blocks.index((MT - 1, qt))
            pearliest[qt] = VT * bi_last + 30
        pdeadline = {0: 160, 1: 184, 2: 218}
        for qt in range(NT - 1):
            for i, (mt, n) in enumerate(
                    [(mt, n) for mt in range(PT) for n in range(2)]):
                units.append((pearliest[qt], pdeadline[qt] + 2 * i,
                              (lambda q_, m_, n_: lambda: proj_chunk(
                                  q_, m_, n_))(qt, mt, n)))
        # qt=3 part A: kk 0-1 need evict(0,3) (s2=222) / evict(1,3) (s2=238)
        for i, (mt, n) in enumerate(
                [(mt, n) for mt in range(PT) for n in range(2)]):
            units.append((238, 240 + 2 * i,
                          (lambda m_, n_: lambda: proj_a(3, m_, n_))(mt, n)))
        # qt=3 part B: tail only (needs the final evict)
        for mt in range(PT):
            for n in range(2):
                units.append((10 ** 9, 10 ** 9,
                              (lambda m_, n_: lambda: proj_b(3, m_, n_))(mt, n)))

        units.sort(key=lambda u: u[1])
        n_stream_units = sum(1 for u in units if u[1] < 10 ** 8)
        NSLOT = NBLK * VT

        # upfront: k(0,0) + q(0,0)
        for h in (0, 1):
            qkv_chunk(wk_s, kT_s, "k", 0, 0, h)
            qkv_chunk(wq_s, qT_s, "q", 0, 0, h)

        pv_ptr = [0]

        def emit_pv_upto(limit):
            while pv_ptr[0] <= limit:
                pbi, pkl = divmod(pv_ptr[0], VT)
                pv_slot(pbi, pkl)
                if pkl == VT - 1:
                    pv_evict(pbi)
                pv_ptr[0] += 1

        emitted = [0]

        def emit_fillers(s):
            # 1) everything overdue (deadline <= s), in deadline order;
            # 2) then pull eligible units forward to the linear ramp so late
            #    slack (proj columns) spreads instead of bunching
            i = 0
            while i < len(units):
                e0, d0, fn = units[i]
                if d0 <= s:
                    assert e0 <= s, f"unit overdue before eligible: {e0} {d0} {s}"
                    units.pop(i)
                    fn()
                    emitted[0] += 1
                else:
                    i += 1
            ramp = ((s + 1) * n_stream_units + NSLOT - 1) // NSLOT
            i = 0
            while emitted[0] < ramp and i < len(units):
                e0, d0, fn = units[i]
                if e0 <= s:
                    units.pop(i)
                    fn()
                    emitted[0] += 1
                else:
                    i += 1

        # Slots are emitted in PAIRS of score matmuls (the second pair's
        # LDWEIGHTS hide inside the first's stream since the 64-row tile
        # config doesn't change), with pv matmuls BATCHED once per PVB slots
        # so the PE pays the row/col-tile config-switch tax (~107ns exposed
        # LDWEIGHTS + drain) once per batch instead of once per pair.
        # den_reduce(bi) is deferred into the next block (kt==2), emitted
        # BEFORE the pair's scores: its matmuls' deps are ready (block ended
        # 2 slots ago) and its reciprocals land at the DVE queue head,
        # releasing the mm psum bufs quickly. The evict that consumes rbc is
        # emitted in the pv batch at s2 = VT*bi + 30, leaving ~12 slots for
        # the den-MM -> recip -> broadcast -> dma chain.
        for s2 in range(0, NBLK * VT, 2):
            for s in (s2, s2 + 1):
                bi, kt = divmod(s, VT)
                sc_exp(bi, kt)
            if s2 % PVB == 6:
                emit_pv_upto(s2 - PVLAG)
            for s in (s2 - 2, s2 - 1):
                if s >= 0:
                    den_add(s)
            emit_fillers(s2)
            # den matmuls amid the fillers: same full-128-row config, so
            # they extend a filler run instead of opening their own
            # (saving a ~107ns LDW exposure on re-entry)
            if s2 % VT == 2 and s2 >= VT:
                den_reduce(s2 // VT - 1)
            emit_fillers(s2 + 1)
        den_add(NBLK * VT - 2)
        den_add(NBLK * VT - 1)
        # a few always-ready dummies in the free pv-pool slot: the PE would
        # otherwise idle ~1.3us between the last score stream and the final
        # den matmuls (which wait on the last exp -> DVE adds)
        pvd = pv_psum.tile([128, 512], f32, tag="pv", name="tail_warm")
        for _ in range(8):
            nc.tensor.matmul(pvd[:, :], lhsT=wsrc[:, 0:128],
                             rhs=wsrc[:, :], start=True, stop=True)
        den_reduce(NBLK - 1)
        emit_pv_upto(NBLK * VT - 1)
        # bridge the den->recip->broadcast->evict chain (~4us) with dummy
        # matmuls so the PE HAM clock doesn't re-throttle before the final
        # projection burst
        # 20 matmuls (~4.3us warm): the den(15)->recip->broadcast->dma->mul
        # chain is ~6.8us past the last exp; a shorter bridge lets the PE
        # idle >3.4us, HAM re-throttles, and the 16 part-B matmuls run at
        # 1.2GHz instead of 2.4
        wps2 = mm_psum.tile([128, 512], f32, tag="mm", name="warm_ps2")
        for _ in range(20):
            nc.tensor.matmul(wps2[:, :], lhsT=wsrc[:, 0:128],
                             rhs=wsrc[:, :], start=True, stop=True)
        # tail: anything left (proj of the last column)
        for _, _, fn in units:
            fn()

        if debug:
            nc.sync.dma_start(out=dbg["qT"], in_=qT_s[:, :, :])
            nc.sync.dma_start(out=dbg["kT"], in_=kT_s[:, :, :])
            nc.sync.dma_start(out=dbg["v"], in_=v_s[:, :, :, :])
            nc.sync.dma_start(out=dbg["aoT"], in_=aoT_s[:, :, :])

    nc.compile()
    return nc


def _get_nc():
    if "nc" not in _NC_CACHE:
        _NC_CACHE["nc"] = _build_nc()
    return _NC_CACHE["nc"]


def _prep_inputs(x, w_qkv, w_proj):
    bf16 = ml_dtypes.bfloat16
    x = np.asarray(x, dtype=np.float32)
    w_qkv = np.asarray(w_qkv, dtype=np.float32)
    w_proj = np.asarray(w_proj, dtype=np.float32)

    w3 = w_qkv.reshape(DIM, 3, HEADS, HDIM)
    wp4 = w_proj.reshape(HEADS, HDIM, DIM)

    def wlay(w):
        # [DIM, INNER_C] -> [128p, MT, KD, 128c] so each on-device DMA row
        # (per partition, per m) is 2KB contiguous
        return np.ascontiguousarray(
            w.reshape(KD, 128, MT, 128).transpose(1, 2, 0, 3)).astype(bf16)

    in_maps = []
    for c in range(NCORES):
        b, hg = c // 2, c % 2
        hs = slice(hg * H_CORE, (hg + 1) * H_CORE)
        wpm = wp4[hs].reshape(INNER_C, DIM)
        in_maps.append({
            "xT": np.ascontiguousarray(x[b].T).astype(bf16),
            "wq": wlay(w3[:, 0, hs].reshape(DIM, INNER_C)),
            "wk": wlay(w3[:, 1, hs].reshape(DIM, INNER_C)),
            "wv": wlay(w3[:, 2, hs].reshape(DIM, INNER_C)),
            "wp": np.ascontiguousarray(
                wpm.reshape(PT, 128, DIM).transpose(1, 0, 2)).astype(bf16),
        })
    return in_maps


def _ensure_trace_hooks():
    """run_bass_kernel_spmd(trace=True) under axon needs antenv.axon_hooks;
    some images lack it. Install a working shim if possible, else make the
    trace path a no-op so execution never crashes on a missing module."""
    import os
    import sys
    try:
        from antenv.axon_hooks import get_axon_ntff_profile_hook  # noqa: F401
        return
    except ImportError:
        pass
    try:
        import types
        from trn_agent_boot.trn_boot import _ntff_profile_via_ctypes

        mod = types.ModuleType("antenv.axon_hooks")
        _h = [_ntff_profile_via_ctypes("/opt/axon/libaxon_pjrt.so")]
        mod.set_axon_ntff_profile_hook = lambda h: _h.__setitem__(0, h)
        mod.get_axon_ntff_profile_hook = lambda: _h[0]
        sys.modules["antenv.axon_hooks"] = mod
        from concourse import bass_utils
        bass_utils.upload_artifacts = lambda tmpdir: tmpdir
    except Exception:
        os.environ["BASS_NEVER_TRACE"] = "1"


def kernel(x, w_qkv, w_proj, b_proj):
    _ensure_trace_hooks()
    from concourse.bass_utils import run_bass_kernel_spmd

    nc = _get_nc()
    in_maps = _prep_inputs(x, w_qkv, w_proj)
    res = run_bass_kernel_spmd(nc, in_maps, core_ids=list(range(NCORES)))
    b_proj = np.asarray(b_proj, dtype=np.float32)
    out = np.empty((B, N, DIM), dtype=np.float32)
    for b in range(B):
        out[b] = (res.results[2 * b]["out"].astype(np.float32)
                  + res.results[2 * b + 1]["out"].astype(np.float32) + b_proj)
    return out

